# revision 1
# baseline (speedup 1.0000x reference)
"""Trainium2 Bass kernel for nn_BaselineGNN (GNN message passing).

Strategy (8 NeuronCores, SPMD):
  - Node-partition the graph: core c owns dst nodes [c*12500, (c+1)*12500).
  - Edges live on the core that owns their dst, grouped by 128-node dst block
    and by src quarter (m-table split in 4 so row ids fit int16 for the
    dma_gather ucode).  Self-loops are NOT materialized as edges; they are a
    single identity matmul per block from the SBUF-resident m tile.
  - Phase 0: global-context encoder u' = relu(gf@Wg+bg)@Wc2+bc (tiny,
    computed redundantly on every core).
  - Phase 1 (own slice): h0 = relu(x@Wc1 + u'[batch]); m = relu(h0@Wm+bm)
    stored fp16; h0^T kept in SBUF; u'[batch] expanded with dma_gather.
  - AllGather m (fp16) so every core can gather any src row.
  - Phase 2: per dst block, aggr^T accumulates in PSUM as
      m_block^T (self loops, via identity matmul)
      + sum_{quarter,chunk} m_gathered[128e,128h]^T @ onehot(dst)[128e,128d]
      + Ws^T @ h0^T
    then h^T = relu(aggr^T + bs), out^T = Wo^T@h^T + bo.
  - All gathers use the dma_gather SWDGE ucode on 4 rotating queues (the 4
    Q7 core-pairs generate descriptors concurrently; ~2.3 ns/row vs ~8 with
    indirect_dma_start).  Trailing -1 indices are skipped by the ucode, so
    chunk padding costs no descriptor-generation time.
"""
import contextlib
import ctypes
import os
import sys

sys.path.insert(0, "/opt/trn_rl_repo")

import numpy as np

import concourse.bass as bass
import concourse.bacc as bacc
import concourse.tile as tile
from concourse import mybir
from concourse.library_config import mlp
from concourse.masks import make_identity

N_NODES, N_EDGES, N_GRAPHS = 100000, 1600000, 1024
IN_LOCAL, IN_GLOBAL, HIDDEN, NUM_CLASSES = 16, 8, 128, 2
P = 128
N_CORES = 8
SLICE = N_NODES // N_CORES            # 12500
NBLK = -(-SLICE // P)                 # 98
PAD_SLICE = NBLK * P                  # 12544
GBLK = N_GRAPHS // P                  # 8
NQUART = 4
QROWS = N_CORES * PAD_SLICE // NQUART  # 25088 (< 2**15, int16-addressable)
UGRP = 7                               # phase-1 groups
UBLK = NBLK // UGRP                    # 14 blocks per group
UIDX = UBLK * P                        # 1792 u-gather idxs per call

f32 = mybir.dt.float32
f16 = mybir.dt.float16
i16 = mybir.dt.int16

_prog_cache: dict = {}
last_run: dict = {}


# --------------------------------------------------------------------------
# device program
# --------------------------------------------------------------------------
def _build(kq, k_used):
    """kq: chunk capacity per (block, quarter); k_used[b][q]: chunks with any
    real edges on any core (static schedule, shared by all cores)."""
    ncalls = NBLK * NQUART
    nchunk_cols = ncalls * kq          # dst one-hot columns
    ix_cols = ncalls * kq * 8          # int16 idx cols (kq*128/16 per call)

    nc = bacc.Bacc("TRN2", target_bir_lowering=False, debug=False,
                   num_devices=N_CORES, num_swdge_queues=4)

    def inp(name, shape, dt):
        return nc.dram_tensor(name, shape, dt, kind="ExternalInput").ap()

    xT_d = inp("xT", [IN_LOCAL, PAD_SLICE], f32)
    gfT_d = inp("gfT", [IN_GLOBAL, N_GRAPHS], f32)
    ixu_d = inp("ixu", [P, UGRP * UIDX // 16], i16)
    ixe_d = inp("ixe", [P, ix_cols], i16)
    dstT_d = inp("dstT", [P, nchunk_cols], f16)
    iota_d = inp("iota", [P, NQUART * kq * P], f16)
    Wg_d = inp("Wg", [IN_GLOBAL, HIDDEN], f32)
    Wc1_d = inp("Wc1", [IN_LOCAL, HIDDEN], f32)
    Wc2_d = inp("Wc2", [HIDDEN, HIDDEN], f32)
    Wm_d = inp("Wm", [HIDDEN, HIDDEN], f32)
    Ws_d = inp("Ws", [HIDDEN, HIDDEN], f32)
    Wo_d = inp("Wo", [HIDDEN, NUM_CLASSES], f32)
    bg_d = inp("bg_c", [HIDDEN, 1], f32)
    bc_d = inp("bc_b", [P, HIDDEN], f32)
    bm_d = inp("bm_b", [P, HIDDEN], f32)
    bs_d = inp("bs_c", [HIDDEN, 1], f32)
    bo_d = inp("bo_c", [NUM_CLASSES, 1], f32)
    id16_d = inp("id16", [P, P], f16)
    cnt_d = inp("cnt", [1, ncalls], mybir.dt.int32)
    outT_d = nc.dram_tensor("outT", [NUM_CLASSES, SLICE], f32,
                            kind="ExternalOutput").ap()

    u_buf = nc.dram_tensor("u_buf", [N_GRAPHS, HIDDEN], f32).ap()
    m_slice = nc.dram_tensor("m_slice", [PAD_SLICE, HIDDEN], f16).ap()
    m_full = nc.dram_tensor("m_full", [N_CORES * PAD_SLICE, HIDDEN], f16,
                            addr_space="Shared").ap()

    AF = mybir.ActivationFunctionType
    OP = mybir.AluOpType

    with tile.TileContext(nc) as tc:
        with (
            tc.tile_pool(name="const", bufs=1) as cpool,
            tc.tile_pool(name="persist", bufs=1) as ppool,
            tc.tile_pool(name="work", bufs=3) as wpool,
            tc.tile_pool(name="sbig", bufs=3) as spool,
            tc.tile_pool(name="uexp", bufs=2) as upool,
            tc.tile_pool(name="xg", bufs=2) as xgpool,
            tc.tile_pool(name="mg", bufs=10) as mgpool,
            tc.tile_pool(name="ps_a", bufs=2, space="PSUM") as ps_a,
            tc.tile_pool(name="ps_b", bufs=3, space="PSUM") as ps_b,
            tc.tile_pool(name="ps_t", bufs=2, space="PSUM") as ps_t,
            tc.tile_pool(name="ps_o", bufs=1, space="PSUM") as ps_o,
        ):
            nc.gpsimd.load_library(mlp)

            def ctile(name, ap, shape, dt):
                t = cpool.tile(shape, dt, tag=f"c_{name}")
                nc.sync.dma_start(t[:], ap[:])
                return t

            Wg_t = ctile("Wg", Wg_d, [IN_GLOBAL, HIDDEN], f32)
            Wc1_t = ctile("Wc1", Wc1_d, [IN_LOCAL, HIDDEN], f32)
            Wc2_t = ctile("Wc2", Wc2_d, [HIDDEN, HIDDEN], f32)
            Wm_t = ctile("Wm", Wm_d, [HIDDEN, HIDDEN], f32)
            Ws_t = ctile("Ws", Ws_d, [HIDDEN, HIDDEN], f32)
            Wo_t = ctile("Wo", Wo_d, [HIDDEN, NUM_CLASSES], f32)
            bg_t = ctile("bg", bg_d, [HIDDEN, 1], f32)
            bc_t = ctile("bc", bc_d, [P, HIDDEN], f32)
            bm_t = ctile("bm", bm_d, [P, HIDDEN], f32)
            bs_t = ctile("bs", bs_d, [HIDDEN, 1], f32)
            bo_t = ctile("bo", bo_d, [NUM_CLASSES, 1], f32)
            gfT_t = ctile("gfT", gfT_d, [IN_GLOBAL, N_GRAPHS], f32)
            id16_t = ctile("id16", id16_d, [P, P], f16)
            iota_t = ctile("iota", iota_d, [P, NQUART * kq * P], f16)

            ident = cpool.tile([P, P], f32)
            make_identity(nc, ident[:])

            cnt_t = ppool.tile([1, ncalls], mybir.dt.int32)
            nc.sync.dma_start(cnt_t[:], cnt_d[:])
            creg = nc.alloc_register(mybir.EngineType.Pool, "gcnt")
            ixu_t = ppool.tile([P, UGRP * UIDX // 16], i16)
            nc.sync.dma_start(ixu_t[:], ixu_d[:])
            ixe_t = ppool.tile([P, ix_cols], i16)
            nc.sync.dma_start(ixe_t[:], ixe_d[:])
            dstT_t = ppool.tile([P, nchunk_cols], f16)
            nc.sync.dma_start(dstT_t[:], dstT_d[:])

            h0T_t = ppool.tile([HIDDEN, PAD_SLICE], f32)    # 6.4 MB persistent
            m16_t = ppool.tile([P, PAD_SLICE], f16)         # 3.2 MB persistent

            # ---------------- phase 0: global encoder ----------------
            for g in range(GBLK):
                gsl = slice(g * P, (g + 1) * P)
                ps1 = ps_b.tile([P, P], f32, tag="pb")
                nc.tensor.matmul(out=ps1[:], lhsT=Wg_t[:], rhs=gfT_t[:, gsl],
                                 start=True, stop=True)
                rT = wpool.tile([P, P], f32, tag="rT")
                nc.scalar.activation(out=rT[:], in_=ps1[:], func=AF.Relu,
                                     bias=bg_t[:, :1])
                ps2 = ps_b.tile([P, P], f32, tag="pb")
                nc.tensor.matmul(out=ps2[:], lhsT=Wc2_t[:], rhs=rT[:],
                                 start=True, stop=True)
                uT = wpool.tile([P, P], f32, tag="uT")
                nc.vector.tensor_copy(out=uT[:], in_=ps2[:])
                ps3 = ps_t.tile([P, P], f32, tag="pt")
                nc.tensor.transpose(out=ps3[:], in_=uT[:], identity=ident[:])
                ub = wpool.tile([P, P], f32, tag="ublk")
                nc.vector.tensor_tensor(out=ub[:], in0=ps3[:], in1=bc_t[:],
                                        op=OP.add)
                nc.sync.dma_start(u_buf[gsl, :], ub[:])

            # ---------------- phase 1: h0 / m on own slice ----------------
            for uc in range(UGRP):
                xg = xgpool.tile([IN_LOCAL, UBLK * P], f32, tag="xg")
                nc.sync.dma_start(xg[:], xT_d[:, uc * UBLK * P:(uc + 1) * UBLK * P])
                uexp = upool.tile([P, UBLK, HIDDEN], f32, tag="uexp")
                nc.gpsimd.dma_gather(
                    uexp[:], u_buf[:], ixu_t[:, uc * (UIDX // 16):(uc + 1) * (UIDX // 16)],
                    UIDX, UIDX, HIDDEN, single_packet=False, queue_num=uc % 4)
                for j in range(UBLK):
                    b = uc * UBLK + j
                    bsl = slice(b * P, (b + 1) * P)
                    psh = ps_b.tile([P, P], f32, tag="pb")
                    nc.tensor.matmul(out=psh[:], lhsT=Wc1_t[:],
                                     rhs=xg[:, j * P:(j + 1) * P],
                                     start=True, stop=False)
                    nc.tensor.matmul(out=psh[:], lhsT=uexp[:, j, :],
                                     rhs=ident[:], is_transpose=True,
                                     start=False, stop=True)
                    nc.vector.tensor_scalar_max(out=h0T_t[:, bsl],
                                                in0=psh[:], scalar1=0.0)
                    psm = ps_b.tile([P, P], f32, tag="pb")
                    nc.tensor.matmul(out=psm[:], lhsT=h0T_t[:, bsl], rhs=Wm_t[:],
                                     start=True, stop=True)
                    nc.vector.tensor_tensor(out=m16_t[:, bsl], in0=psm[:],
                                            in1=bm_t[:], op=OP.add)
                    nc.vector.tensor_scalar_max(out=m16_t[:, bsl],
                                                in0=m16_t[:, bsl], scalar1=0.0)
                    nc.sync.dma_start(m_slice[bsl, :], m16_t[:, bsl])

            # ---------------- allgather m ----------------
            if not os.environ.get("GNN_NO_CC"):
                nc.gpsimd.collective_compute(
                    "AllGather", OP.bypass,
                    replica_groups=[list(range(N_CORES))],
                    ins=[m_slice[:]], outs=[m_full[:]])

            # ---------------- phase 2: scatter-add + update + readout ------
            iota_v = iota_t[:].rearrange("p (k f) -> p k f", k=NQUART * kq)
            ixc = kq * 8   # idx cols per call
            for b in range(NBLK):
                bsl = slice(b * P, (b + 1) * P)
                csl = slice(b * NQUART * kq, (b + 1) * NQUART * kq)
                S = spool.tile([P, NQUART * kq, P], f16, tag="S")
                nc.vector.tensor_tensor(
                    out=S[:],
                    in0=dstT_t[:, csl].to_broadcast([P, NQUART * kq, P]),
                    in1=iota_v, op=OP.is_equal)
                pa = ps_a.tile([HIDDEN, P], f32, tag="pa")
                # self loops: aggrT += m_block^T
                nc.tensor.matmul(out=pa[:], lhsT=m16_t[:, bsl], rhs=id16_t[:],
                                 start=True, stop=False)
                for q in range(NQUART):
                    call = b * NQUART + q
                    mg = mgpool.tile([P, kq, HIDDEN], f16, tag="mg")
                    if os.environ.get("GNN_NO_EG"):
                        nc.gpsimd.memset(mg[:], 0)
                    else:
                        nc.gpsimd.reg_load(creg, cnt_t[0:1, call:call + 1])
                        nc.gpsimd.dma_gather(
                            mg[:], m_full[q * QROWS:(q + 1) * QROWS, :],
                            ixe_t[:, call * ixc:(call + 1) * ixc],
                            kq * P, creg, HIDDEN,
                            single_packet=False, queue_num=q)
                    for k in range(k_used[b][q]):
                        if os.environ.get("GNN_NO_CMM"): break
                        nc.tensor.matmul(out=pa[:], lhsT=mg[:, k, :],
                                         rhs=S[:, q * kq + k, :],
                                         start=False, stop=False)
                nc.tensor.matmul(out=pa[:], lhsT=Ws_t[:], rhs=h0T_t[:, bsl],
                                 start=False, stop=True)
                hT = wpool.tile([HIDDEN, P], f32, tag="hT")
                nc.scalar.activation(out=hT[:], in_=pa[:], func=AF.Relu,
                                     bias=bs_t[:, :1])
                po = ps_o.tile([NUM_CLASSES, P], f32, tag="po")
                nc.tensor.matmul(out=po[:], lhsT=Wo_t[:], rhs=hT[:],
                                 start=True, stop=True)
                ob = wpool.tile([NUM_CLASSES, P], f32, tag="ob")
                nc.scalar.activation(out=ob[:], in_=po[:],
                                     func=AF.Identity, bias=bo_t[:, :1])
                w = min(SLICE, (b + 1) * P) - b * P
                nc.sync.dma_start(outT_d[:, b * P:b * P + w], ob[:, :w])

    nc.compile()
    return nc


# --------------------------------------------------------------------------
# host side
# --------------------------------------------------------------------------
def _wrap16(ix):
    """dma_gather int16 index layout: [16, n/16] wrapped, tiled to 128 parts."""
    return np.tile(ix.reshape(-1, 16).T, (8, 1))


def _preprocess(inputs):
    x = np.asarray(inputs["x"], dtype=np.float32)
    ei = np.asarray(inputs["edge_index"]).astype(np.int64)
    batch = np.asarray(inputs["batch"]).astype(np.int64)
    gf = np.asarray(inputs["global_feat"], dtype=np.float32)
    W = {k: np.ascontiguousarray(np.asarray(inputs[k], dtype=np.float32))
         for k in ("Wg", "bg", "Wc", "bc", "Wm", "bm", "Ws", "bs", "Wo", "bo")}

    src_all, dst_all = ei[0], ei[1]
    src_row = (src_all // SLICE) * PAD_SLICE + (src_all % SLICE)
    quarter = src_row // QROWS
    loc16 = (src_row % QROWS).astype(np.int16)
    core_of = dst_all // SLICE

    per_core = []
    counts = np.zeros((N_CORES, NBLK, NQUART), np.int64)
    for c in range(N_CORES):
        sel = np.nonzero(core_of == c)[0]
        d_loc = dst_all[sel] - c * SLICE
        blk = d_loc // P
        q = quarter[sel]
        key = blk * NQUART + q
        order = np.argsort(key, kind="stable")
        sel, key = sel[order], key[order]
        cnt = np.bincount(key, minlength=NBLK * NQUART).reshape(NBLK, NQUART)
        counts[c] = cnt
        per_core.append((sel, (d_loc[order] % P).astype(np.float16),
                         loc16[sel], cnt))

    kq = int(-(-counts.max() // P))
    k_used = (-(-counts.max(axis=0) // P)).astype(np.int64)  # [NBLK, NQUART]
    cap = kq * P

    iota_np = np.tile(np.arange(P, dtype=np.float16), (P, NQUART * kq))
    shared = {
        "gfT": np.ascontiguousarray(gf.T),
        "iota": iota_np,
        "id16": np.eye(P, dtype=np.float16),
        "Wg": W["Wg"],
        "Wc1": np.ascontiguousarray(W["Wc"][:IN_LOCAL]),
        "Wc2": np.ascontiguousarray(W["Wc"][IN_LOCAL:]),
        "Wm": W["Wm"], "Ws": W["Ws"], "Wo": W["Wo"],
        "bg_c": W["bg"].reshape(HIDDEN, 1),
        "bc_b": np.tile(W["bc"], (P, 1)),
        "bm_b": np.tile(W["bm"], (P, 1)),
        "bs_c": W["bs"].reshape(HIDDEN, 1),
        "bo_c": W["bo"].reshape(NUM_CLASSES, 1),
    }

    in_maps = []
    for c in range(N_CORES):
        sel, d128, l16, cnt = per_core[c]
        ncalls = NBLK * NQUART
        ix_pad = np.full(ncalls * cap, -1, np.int16)
        # first 8 calls: pad with a valid row (0) so the first use of every
        # mg pool slot writes the whole tile (NaN hygiene for stale reads)
        ix_pad[:10 * cap] = 0
        dst_pad = np.full(ncalls * cap, -1.0, np.float16)
        flat_cnt = cnt.reshape(-1)
        cum = np.cumsum(flat_cnt) - flat_cnt
        within = np.arange(len(sel)) - np.repeat(cum, flat_cnt)
        pos = np.repeat(np.arange(ncalls) * cap, flat_cnt) + within
        ix_pad[pos] = l16
        dst_pad[pos] = d128
        cnt_call = flat_cnt.astype(np.int32).copy()
        cnt_call[:10] = cap          # first 6 calls are 0-padded to full
        ixe = np.concatenate(
            [_wrap16(ix_pad[i * cap:(i + 1) * cap]) for i in range(ncalls)],
            axis=1)
        # dst one-hot source: col (call*kq + k), partition p = edge k*128+p
        dstT = np.ascontiguousarray(dst_pad.reshape(-1, P).T)

        bpad = np.zeros(PAD_SLICE, np.int16)
        bpad[:SLICE] = batch[c * SLICE:(c + 1) * SLICE]
        ixu = np.concatenate(
            [_wrap16(bpad[g * UIDX:(g + 1) * UIDX]) for g in range(UGRP)],
            axis=1)

        xT = np.zeros((IN_LOCAL, PAD_SLICE), np.float32)
        xT[:, :SLICE] = x[c * SLICE:(c + 1) * SLICE].T

        m = dict(shared)
        m.update({"xT": xT, "ixu": ixu, "ixe": ixe, "dstT": dstT,
                  "cnt": cnt_call[None, :]})
        in_maps.append(m)
    return kq, k_used, in_maps


# --------------------------------------------------------------------------
# profiling hook (NTFF via the axon PJRT .so; absent module in this image)
# --------------------------------------------------------------------------
def _profile_hook():
    so = "/opt/axon/libaxon_pjrt.so"
    if not os.path.exists(so):
        return None
    lib = ctypes.CDLL(so)
    if not hasattr(lib, "axon_start_nrt_profile"):
        return None
    lib.axon_start_nrt_profile.argtypes = [ctypes.POINTER(ctypes.c_int64),
                                           ctypes.c_size_t]
    lib.axon_start_nrt_profile.restype = ctypes.c_int64
    lib.axon_stop_nrt_profile.argtypes = [ctypes.c_char_p]
    lib.axon_stop_nrt_profile.restype = ctypes.c_int64

    @contextlib.contextmanager
    def hook(output_dir, device_ids):
        import jax
        jax.devices()
        if device_ids:
            ids = (ctypes.c_int64 * len(device_ids))(*device_ids)
            rc = lib.axon_start_nrt_profile(ids, len(device_ids))
        else:
            rc = lib.axon_start_nrt_profile(None, 0)
        if rc != 0:
            raise RuntimeError(f"axon_start_nrt_profile rc={rc}")
        try:
            yield
        finally:
            n = lib.axon_stop_nrt_profile(str(output_dir).encode())
            print(f"profile: {n} file(s) written to {output_dir}",
                  file=sys.stderr)

    return hook


def _run(nc, in_maps):
    from concourse import bass2jax
    trace_dir = os.environ.get("GNN_TRACE_DIR", "")
    if not trace_dir:
        return bass2jax.run_bass_via_pjrt(nc, in_maps, n_cores=N_CORES)
    hook = _profile_hook()
    if hook is None:
        return bass2jax.run_bass_via_pjrt(nc, in_maps, n_cores=N_CORES)
    import time as _time
    trace_dir = os.path.join(trace_dir, f"run_{int(_time.time()*1000)}")
    os.makedirs(trace_dir, exist_ok=True)
    last_run["trace_dir"] = trace_dir
    trace_cores = [int(t) for t in
                   os.environ.get("GNN_TRACE_CORES", "0").split(",")]
    with hook(trace_dir, trace_cores):
        results = bass2jax.run_bass_via_pjrt(nc, in_maps, n_cores=N_CORES)
    try:
        from concourse._compat import FishPath
        import gauge.profiler as gprof
        profile = gprof.Profile(
            profile_path=FishPath(trace_dir), kernel_dev_mode=True,
            profile_on_exit=False, bass_kernel=nc.m,
            offline_processing=True, fname="*_body*")
        profile.convert_ntffs_to_json(tuple(trace_cores))
        j = profile.load_json(trace_cores[0])
        last_run["summary"] = j["summary"][0] if j else None
        last_run["exec_time_ns"] = (
            int(j["summary"][0]["total_time"] * 1e9) if j else None)
        last_run["profile_json"] = str(profile.json_path(trace_cores[0]))
    except Exception as e:  # profiling must never break the run
        print(f"profile post-processing failed: {e}", file=sys.stderr)
    return results


def kernel(**inputs) -> np.ndarray:
    kq, k_used, in_maps = _preprocess(inputs)
    key = (kq, k_used.tobytes())
    nc = _prog_cache.get(key)
    if nc is None:
        nc = _build(kq, k_used)
        _prog_cache[key] = nc
    last_run.clear()
    last_run["kq"] = kq
    results = _run(nc, in_maps)
    outT = np.concatenate([results[c]["outT"] for c in range(N_CORES)], axis=1)
    return np.ascontiguousarray(outT.T.astype(np.float32))



# revision 10
# speedup vs baseline: 1.0694x; 1.0694x over previous
"""Trainium2 Bass kernel for nn_BaselineGNN (GNN message passing).

Strategy (8 NeuronCores, SPMD), v3:
  - Node-partition: core c owns dst nodes [c*12500, (c+1)*12500).
  - Phase 0: core c only needs graphs [g_lo*128, g_lo*128+256) (batch is
    sorted); host slices gfT per core.  u'T [128H, 256] computed with 2
    matmuls + fused bias/relu activations, PE-transposed into SBUF rows.
  - Phase 1: h0T/mT [128, 12544] f16 via 512-wide matmuls; u'[batch]
    added with 2 one-hot matmuls against a host-built batch-onehot B
    (streamed per chunk); biases+relu fused into scalar activations.
    mT is PE-transposed per block and DMA'd to m_slice, then ONE
    AllGather -> m_full [100352, 128] f16 (core-major rows); quarter
    tables are int16-addressable views.
  - Phase 2: per dst block, aggrT accumulates in PSUM as
      ident @ mT_block  (self loops)
      + sum_{quarter,chunk} m_gathered[128e,128h]^T @ onehot(dst)[128e,128d]
      + Ws^T @ h0T_block
    then hT = relu(aggrT + bs), out = Wo^T @ hT + bo.
  - Gathers: one SWDGE call per (5-block group x quarter), mg pools
    bufs=3, queue q per quarter.  The gpsimd engine runs ONLY the gather
    stream (library load + AllGather issued before it), so the 4 SWDGE
    queues' descriptor generation runs concurrently (~5 ns/row/queue);
    the engine blocking on its own queue is harmless.
  - S one-hots are built with a uniform-shape is_equal per block (dstT
    padded to KbMax chunks/block) to keep the DVE 2x mode.
"""
import contextlib
import ctypes
import os
import sys

sys.path.insert(0, "/opt/trn_rl_repo")

import numpy as np

import concourse.bass as bass
import concourse.bacc as bacc
import concourse.tile as tile
from concourse import mybir
from concourse.library_config import mlp
from concourse.masks import make_identity

N_NODES, N_EDGES, N_GRAPHS = 100000, 1600000, 1024
IN_LOCAL, IN_GLOBAL, HIDDEN, NUM_CLASSES = 16, 8, 128, 2
P = 128
N_CORES = 8
SLICE = N_NODES // N_CORES            # 12500
NBLK = -(-SLICE // P)                 # 98
PAD_SLICE = NBLK * P                  # 12544
NQ = 4                                # quarter tables (int16 addressing)
QROWS = N_CORES * PAD_SLICE // NQ     # 25088
GRP = 5                               # dst blocks per gather group
NGRP = -(-NBLK // GRP)                # 20 (last group ragged: 3 blocks)
CHW = 512                             # phase-1 column chunk

f32 = mybir.dt.float32
f16 = mybir.dt.float16
i16 = mybir.dt.int16

_prog_cache: dict = {}
last_run: dict = {}


def _grp_blocks(g):
    return range(g * GRP, min((g + 1) * GRP, NBLK))


# --------------------------------------------------------------------------
# device program
# --------------------------------------------------------------------------
def _build(k_used):
    """k_used[b][q]: chunks for dst block b from quarter q (shared)."""
    Kb = k_used.sum(axis=1)
    KbMax = int(Kb.max())
    qoff = np.cumsum(k_used, axis=1) - k_used      # within-block chunk offset
    K_gq = np.zeros((NGRP, NQ), np.int64)
    for g in range(NGRP):
        for q in range(NQ):
            K_gq[g][q] = sum(k_used[b][q] for b in _grp_blocks(g))
    KCAP = [int(K_gq[:, q].max()) for q in range(NQ)]
    gcols = [int(K_gq[g].sum()) * 8 for g in range(NGRP)]
    GCOLS_MAX = max(gcols)
    total_ix_cols = sum(gcols)

    nc = bacc.Bacc("TRN2", target_bir_lowering=False, debug=False,
                   num_devices=N_CORES, num_swdge_queues=4)

    def inp(name, shape, dt):
        return nc.dram_tensor(name, shape, dt, kind="ExternalInput").ap()

    xT_d = inp("xT", [IN_LOCAL, PAD_SLICE], f16)
    gfT_d = inp("gfT", [IN_GLOBAL, 2 * P], f16)
    B_d = inp("B", [2, P, PAD_SLICE], f16)
    ixe_d = inp("ixe", [P, total_ix_cols], i16)
    dstT_d = inp("dstT", [P, NBLK * KbMax], f16)
    iota_d = inp("iota", [P, KbMax * P], f16)
    Wg_d = inp("Wg", [IN_GLOBAL, HIDDEN], f16)
    Wc1_d = inp("Wc1", [IN_LOCAL, HIDDEN], f16)
    Wc2_d = inp("Wc2", [HIDDEN, HIDDEN], f16)
    Wm_d = inp("Wm", [HIDDEN, HIDDEN], f16)
    Ws_d = inp("Ws", [HIDDEN, HIDDEN], f16)
    Wo_d = inp("Wo", [HIDDEN, NUM_CLASSES], f16)
    bg_d = inp("bg_c", [HIDDEN, 1], f32)
    bc_d = inp("bc_c", [HIDDEN, 1], f32)
    bm_d = inp("bm_c", [HIDDEN, 1], f32)
    bs_d = inp("bs_c", [HIDDEN, 1], f32)
    bo_d = inp("bo_c", [NUM_CLASSES, 1], f32)
    id16_d = inp("id16", [P, P], f16)
    outT_d = nc.dram_tensor("outT", [NUM_CLASSES, SLICE], f32,
                            kind="ExternalOutput").ap()

    m_slice = nc.dram_tensor("m_slice", [PAD_SLICE, HIDDEN], f16).ap()
    m_full = nc.dram_tensor("m_full", [N_CORES * PAD_SLICE, HIDDEN], f16,
                            addr_space="Shared").ap()

    AF = mybir.ActivationFunctionType
    OP = mybir.AluOpType

    with tile.TileContext(nc) as tc:
        with (
            tc.tile_pool(name="const", bufs=1) as cpool,
            tc.tile_pool(name="persist", bufs=1) as ppool,
            tc.tile_pool(name="work", bufs=3) as wpool,
            tc.tile_pool(name="bB", bufs=3) as bpool,
            tc.tile_pool(name="ix", bufs=2) as ixpool,
            tc.tile_pool(name="sS", bufs=3) as spool,
            tc.tile_pool(name="mg0", bufs=3) as mgp0,
            tc.tile_pool(name="mg1", bufs=3) as mgp1,
            tc.tile_pool(name="mg2", bufs=3) as mgp2,
            tc.tile_pool(name="mg3", bufs=3) as mgp3,
            tc.tile_pool(name="ps_big", bufs=2, space="PSUM") as ps_big,
            tc.tile_pool(name="ps_t", bufs=2, space="PSUM") as ps_t,
            tc.tile_pool(name="ps_a", bufs=3, space="PSUM") as ps_a,
            tc.tile_pool(name="ps_o", bufs=1, space="PSUM") as ps_o,
        ):
            mgpools = [mgp0, mgp1, mgp2, mgp3]
            nc.gpsimd.load_library(mlp)

            def ctile(name, ap, shape, dt):
                t = cpool.tile(shape, dt, tag=f"c_{name}", name=f"c_{name}")
                nc.sync.dma_start(t[:], ap[:])
                return t

            Wg_t = ctile("Wg", Wg_d, [IN_GLOBAL, HIDDEN], f16)
            Wc1_t = ctile("Wc1", Wc1_d, [IN_LOCAL, HIDDEN], f16)
            Wc2_t = ctile("Wc2", Wc2_d, [HIDDEN, HIDDEN], f16)
            Wm_t = ctile("Wm", Wm_d, [HIDDEN, HIDDEN], f16)
            Ws_t = ctile("Ws", Ws_d, [HIDDEN, HIDDEN], f16)
            Wo_t = ctile("Wo", Wo_d, [HIDDEN, NUM_CLASSES], f16)
            bg_t = ctile("bg", bg_d, [HIDDEN, 1], f32)
            bc_t = ctile("bc", bc_d, [HIDDEN, 1], f32)
            bm_t = ctile("bm", bm_d, [HIDDEN, 1], f32)
            bs_t = ctile("bs", bs_d, [HIDDEN, 1], f32)
            bo_t = ctile("bo", bo_d, [NUM_CLASSES, 1], f32)
            gfT_t = ctile("gfT", gfT_d, [IN_GLOBAL, 2 * P], f16)
            id16_t = ctile("id16", id16_d, [P, P], f16)
            iota_t = ctile("iota", iota_d, [P, KbMax * P], f16)
            dstT_t = ctile("dstT", dstT_d, [P, NBLK * KbMax], f16)
            xT_t = ctile("xT", xT_d, [IN_LOCAL, PAD_SLICE], f16)

            h0T_t = ppool.tile([HIDDEN, PAD_SLICE], f16)    # 25 KB/p
            m16_t = ppool.tile([HIDDEN, PAD_SLICE], f16)    # 25 KB/p
            u_rows = ppool.tile([P, 2, P], f16)             # u' rows, 2 blocks

            # ---------------- phase 0: global encoder (256 graphs) ---------
            ps = ps_big.tile([P, CHW], f32, tag="pbig")
            nc.tensor.matmul(out=ps[:, :2 * P], lhsT=Wg_t[:], rhs=gfT_t[:],
                             start=True, stop=True)
            rT = wpool.tile([HIDDEN, 2 * P], f16, tag="rT")
            nc.scalar.activation(out=rT[:], in_=ps[:, :2 * P], func=AF.Relu,
                                 bias=bg_t[:, :1])
            ps = ps_big.tile([P, CHW], f32, tag="pbig")
            nc.tensor.matmul(out=ps[:, :2 * P], lhsT=Wc2_t[:], rhs=rT[:],
                             start=True, stop=True)
            uT = wpool.tile([HIDDEN, 2 * P], f16, tag="uT")
            nc.scalar.activation(out=uT[:], in_=ps[:, :2 * P],
                                 func=AF.Identity, bias=bc_t[:, :1])
            for t in range(2):
                pst = ps_t.tile([P, P], f16, tag="pt")
                nc.tensor.matmul(out=pst[:], lhsT=uT[:, t * P:(t + 1) * P],
                                 rhs=id16_t[:], is_transpose=True,
                                 start=True, stop=True)
                nc.vector.tensor_copy(out=u_rows[:, t, :], in_=pst[:])

            # ---------------- phase 1: h0T / mT ----------------
            for o in range(0, PAD_SLICE, CHW):
                w = min(CHW, PAD_SLICE - o)
                gsl = slice(o, o + w)
                B0 = bpool.tile([P, CHW], f16, tag="B0")
                nc.sync.dma_start(B0[:, :w], B_d[0, :, gsl])
                B1 = bpool.tile([P, CHW], f16, tag="B1")
                nc.sync.dma_start(B1[:, :w], B_d[1, :, gsl])
                ps = ps_big.tile([P, CHW], f32, tag="pbig")
                nc.tensor.matmul(out=ps[:, :w], lhsT=Wc1_t[:],
                                 rhs=xT_t[:, gsl], start=True, stop=False)
                nc.tensor.matmul(out=ps[:, :w], lhsT=u_rows[:, 0, :],
                                 rhs=B0[:, :w], start=False, stop=False)
                nc.tensor.matmul(out=ps[:, :w], lhsT=u_rows[:, 1, :],
                                 rhs=B1[:, :w], start=False, stop=True)
                nc.scalar.activation(out=h0T_t[:, gsl], in_=ps[:, :w],
                                     func=AF.Relu)
                ps2 = ps_big.tile([P, CHW], f32, tag="pbig")
                nc.tensor.matmul(out=ps2[:, :w], lhsT=Wm_t[:],
                                 rhs=h0T_t[:, gsl], start=True, stop=True)
                nc.scalar.activation(out=m16_t[:, gsl], in_=ps2[:, :w],
                                     func=AF.Relu, bias=bm_t[:, :1])
                for b in range(o // P, (o + w) // P):
                    bsl = slice(b * P, (b + 1) * P)
                    pst = ps_t.tile([P, P], f16, tag="pt")
                    nc.tensor.matmul(out=pst[:], lhsT=m16_t[:, bsl],
                                     rhs=id16_t[:], is_transpose=True,
                                     start=True, stop=True)
                    mrow = wpool.tile([P, P], f16, tag="mrow")
                    nc.vector.tensor_copy(out=mrow[:], in_=pst[:])
                    nc.sync.dma_start(m_slice[bsl, :], mrow[:])

            # ---------------- allgather m ----------------
            if not os.environ.get("GNN_NO_CC"):
                nc.gpsimd.collective_compute(
                    "AllGather", OP.bypass,
                    replica_groups=[list(range(N_CORES))],
                    ins=[m_slice[:]], outs=[m_full[:]])

            # ---------------- phase 2: scatter-add + update + readout ------
            iota_v = iota_t[:].rearrange("p (k f) -> p k f", k=KbMax)
            for g in range(NGRP):
                goff = sum(gcols[:g])
                ixg = ixpool.tile([P, GCOLS_MAX], i16, tag="ixg")
                nc.sync.dma_start(ixg[:, :gcols[g]],
                                  ixe_d[:, goff:goff + gcols[g]])
                mg = []
                qo = 0
                for q in range(NQ):
                    kgq = int(K_gq[g][q])
                    mgt = mgpools[q].tile([P, KCAP[q], P], f16, tag=f"mg{q}")
                    mg.append(mgt)
                    if kgq and not os.environ.get("GNN_NO_EG"):
                        nc.gpsimd.dma_gather(
                            mgt[:, :kgq, :],
                            m_full[q * QROWS:(q + 1) * QROWS, :],
                            ixg[:, qo:qo + kgq * 8],
                            kgq * P, kgq * P, HIDDEN,
                            single_packet=False, queue_num=q)
                    qo += kgq * 8
                for b in _grp_blocks(g):
                    bsl = slice(b * P, (b + 1) * P)
                    csl = slice(b * KbMax, (b + 1) * KbMax)
                    S = spool.tile([P, KbMax, P], f16, tag="S")
                    nc.vector.tensor_tensor(
                        out=S[:],
                        in0=dstT_t[:, csl].to_broadcast([P, KbMax, P]),
                        in1=iota_v, op=OP.is_equal)
                    pa = ps_a.tile([HIDDEN, P], f32, tag="pa")
                    nc.tensor.matmul(out=pa[:], lhsT=id16_t[:],
                                     rhs=m16_t[:, bsl], start=True, stop=False)
                    for q in range(NQ):
                        co = int(sum(k_used[b2][q] for b2 in _grp_blocks(g)
                                     if b2 < b))
                        so = int(qoff[b][q])
                        for k in range(int(k_used[b][q])):
                            if os.environ.get("GNN_NO_CMM"):
                                break
                            nc.tensor.matmul(out=pa[:],
                                             lhsT=mg[q][:, co + k, :],
                                             rhs=S[:, so + k, :],
                                             start=False, stop=False)
                    nc.tensor.matmul(out=pa[:], lhsT=Ws_t[:],
                                     rhs=h0T_t[:, bsl], start=False, stop=True)
                    hT = wpool.tile([HIDDEN, P], f16, tag="hT")
                    nc.scalar.activation(out=hT[:], in_=pa[:], func=AF.Relu,
                                         bias=bs_t[:, :1])
                    po = ps_o.tile([NUM_CLASSES, P], f32, tag="po")
                    nc.tensor.matmul(out=po[:], lhsT=Wo_t[:], rhs=hT[:],
                                     start=True, stop=True)
                    ob = wpool.tile([NUM_CLASSES, P], f32, tag="ob")
                    nc.scalar.activation(out=ob[:], in_=po[:],
                                         func=AF.Identity, bias=bo_t[:, :1])
                    w = min(SLICE, (b + 1) * P) - b * P
                    nc.sync.dma_start(outT_d[:, b * P:b * P + w], ob[:, :w])

    nc.compile()
    return nc


# --------------------------------------------------------------------------
# host side
# --------------------------------------------------------------------------
def _wrap16(ix):
    """dma_gather int16 index layout: [16, n/16] wrapped, tiled to 128 parts."""
    return np.tile(ix.reshape(-1, 16).T, (8, 1))


def _preprocess(inputs):
    x = np.asarray(inputs["x"], dtype=np.float32)
    ei = np.asarray(inputs["edge_index"]).astype(np.int64)
    batch = np.asarray(inputs["batch"]).astype(np.int64)
    gf = np.asarray(inputs["global_feat"], dtype=np.float32)
    W = {k: np.ascontiguousarray(np.asarray(inputs[k], dtype=np.float32))
         for k in ("Wg", "bg", "Wc", "bc", "Wm", "bm", "Ws", "bs", "Wo", "bo")}

    src_all, dst_all = ei[0], ei[1]
    src_row = (src_all // SLICE) * PAD_SLICE + (src_all % SLICE)
    quarter = src_row // QROWS
    loc16 = (src_row % QROWS).astype(np.int16)
    core_of = dst_all // SLICE

    per_core = []
    counts = np.zeros((N_CORES, NBLK, NQ), np.int64)
    for c in range(N_CORES):
        sel = np.nonzero(core_of == c)[0]
        d_loc = dst_all[sel] - c * SLICE
        blk = d_loc // P
        q = quarter[sel]
        key = blk * NQ + q
        order = np.argsort(key, kind="stable")
        sel, key = sel[order], key[order]
        cnt = np.bincount(key, minlength=NBLK * NQ).reshape(NBLK, NQ)
        counts[c] = cnt
        per_core.append((sel, (d_loc[order] % P).astype(np.float16),
                         loc16[sel], cnt))

    k_used = (-(-counts.max(axis=0) // P)).astype(np.int64)   # [NBLK, NQ]
    caps = k_used * P
    Kb = k_used.sum(axis=1)
    KbMax = int(Kb.max())
    qoff = np.cumsum(k_used, axis=1) - k_used
    packed_off = np.cumsum(k_used.reshape(-1)) - k_used.reshape(-1)
    packed_off = packed_off.reshape(NBLK, NQ)

    shared = {
        "iota": np.tile(np.arange(P, dtype=np.float16), (P, KbMax)),
        "id16": np.eye(P, dtype=np.float16),
        "Wg": W["Wg"].astype(np.float16),
        "Wc1": np.ascontiguousarray(W["Wc"][:IN_LOCAL]).astype(np.float16),
        "Wc2": np.ascontiguousarray(W["Wc"][IN_LOCAL:]).astype(np.float16),
        "Wm": W["Wm"].astype(np.float16),
        "Ws": W["Ws"].astype(np.float16),
        "Wo": W["Wo"].astype(np.float16),
        "bg_c": W["bg"].reshape(HIDDEN, 1),
        "bc_c": W["bc"].reshape(HIDDEN, 1),
        "bm_c": W["bm"].reshape(HIDDEN, 1),
        "bs_c": W["bs"].reshape(HIDDEN, 1),
        "bo_c": W["bo"].reshape(NUM_CLASSES, 1),
    }

    in_maps = []
    for c in range(N_CORES):
        sel, d128, r16, cnt = per_core[c]
        flat_cnt = cnt.reshape(-1)
        cum = np.cumsum(flat_cnt) - flat_cnt
        within = np.arange(len(sel)) - np.repeat(cum, flat_cnt)
        # dstT: padded layout (b*KbMax + qoff) for uniform S builds
        base_dst = ((np.arange(NBLK)[:, None] * KbMax + qoff) * P).reshape(-1)
        pos_dst = np.repeat(base_dst, flat_cnt) + within
        dst_pad = np.full(NBLK * KbMax * P, -1.0, np.float16)
        dst_pad[pos_dst] = d128
        dstT = np.ascontiguousarray(dst_pad.reshape(-1, P).T)
        # ixe: packed layout
        pos_ix = np.repeat((packed_off.reshape(-1)) * P, flat_cnt) + within
        ix_pad = np.zeros(int(k_used.sum()) * P, np.int16)
        ix_pad[pos_ix] = r16
        parts = []
        for g in range(NGRP):
            for q in range(NQ):
                for b in _grp_blocks(g):
                    o = packed_off[b][q] * P
                    parts.append(ix_pad[o:o + caps[b][q]])
        ixe = _wrap16(np.concatenate(parts))

        bslice = batch[c * SLICE:(c + 1) * SLICE]
        g0 = int(bslice[0])
        grel = bslice - g0
        assert grel.max() < 2 * P, "core slice spans >256 graphs"
        B = np.zeros((2, P, PAD_SLICE), np.float16)
        n_idx = np.arange(SLICE)
        B[grel // P, grel % P, n_idx] = 1.0
        gfT = np.zeros((IN_GLOBAL, 2 * P), np.float16)
        g_hi = min(g0 + 2 * P, N_GRAPHS)
        gfT[:, :g_hi - g0] = gf.T[:, g0:g_hi].astype(np.float16)

        xT = np.zeros((IN_LOCAL, PAD_SLICE), np.float16)
        xT[:, :SLICE] = x[c * SLICE:(c + 1) * SLICE].T

        m = dict(shared)
        m.update({"xT": xT, "ixe": np.ascontiguousarray(ixe), "dstT": dstT,
                  "B": B, "gfT": gfT})
        in_maps.append(m)
    return k_used, in_maps


# --------------------------------------------------------------------------
# profiling hook (NTFF via the axon PJRT .so; absent module in this image)
# --------------------------------------------------------------------------
def _profile_hook():
    so = "/opt/axon/libaxon_pjrt.so"
    if not os.path.exists(so):
        return None
    lib = ctypes.CDLL(so)
    if not hasattr(lib, "axon_start_nrt_profile"):
        return None
    lib.axon_start_nrt_profile.argtypes = [ctypes.POINTER(ctypes.c_int64),
                                           ctypes.c_size_t]
    lib.axon_start_nrt_profile.restype = ctypes.c_int64
    lib.axon_stop_nrt_profile.argtypes = [ctypes.c_char_p]
    lib.axon_stop_nrt_profile.restype = ctypes.c_int64

    @contextlib.contextmanager
    def hook(output_dir, device_ids):
        import jax
        jax.devices()
        if device_ids:
            ids = (ctypes.c_int64 * len(device_ids))(*device_ids)
            rc = lib.axon_start_nrt_profile(ids, len(device_ids))
        else:
            rc = lib.axon_start_nrt_profile(None, 0)
        if rc != 0:
            raise RuntimeError(f"axon_start_nrt_profile rc={rc}")
        try:
            yield
        finally:
            n = lib.axon_stop_nrt_profile(str(output_dir).encode())
            print(f"profile: {n} file(s) written to {output_dir}",
                  file=sys.stderr)

    return hook


def _run(nc, in_maps):
    from concourse import bass2jax
    trace_dir = os.environ.get("GNN_TRACE_DIR", "")
    if not trace_dir:
        return bass2jax.run_bass_via_pjrt(nc, in_maps, n_cores=N_CORES)
    hook = _profile_hook()
    if hook is None:
        return bass2jax.run_bass_via_pjrt(nc, in_maps, n_cores=N_CORES)
    import time as _time
    trace_dir = os.path.join(trace_dir, f"run_{int(_time.time()*1000)}")
    os.makedirs(trace_dir, exist_ok=True)
    last_run["trace_dir"] = trace_dir
    trace_cores = [int(t) for t in
                   os.environ.get("GNN_TRACE_CORES", "0").split(",")]
    with hook(trace_dir, trace_cores):
        results = bass2jax.run_bass_via_pjrt(nc, in_maps, n_cores=N_CORES)
    try:
        from concourse._compat import FishPath
        import gauge.profiler as gprof
        profile = gprof.Profile(
            profile_path=FishPath(trace_dir), kernel_dev_mode=True,
            profile_on_exit=False, bass_kernel=nc.m,
            offline_processing=True, fname="*_body*")
        profile.convert_ntffs_to_json(tuple(trace_cores))
        j = profile.load_json(trace_cores[0])
        last_run["summary"] = j["summary"][0] if j else None
        last_run["exec_time_ns"] = (
            int(j["summary"][0]["total_time"] * 1e9) if j else None)
        last_run["profile_json"] = str(profile.json_path(trace_cores[0]))
    except Exception as e:  # profiling must never break the run
        print(f"profile post-processing failed: {e}", file=sys.stderr)
    return results


def kernel(**inputs) -> np.ndarray:
    k_used, in_maps = _preprocess(inputs)
    key = k_used.tobytes()
    nc = _prog_cache.get(key)
    if nc is None:
        nc = _build(k_used)
        _prog_cache[key] = nc
    last_run.clear()
    results = _run(nc, in_maps)
    outT = np.concatenate([results[c]["outT"] for c in range(N_CORES)], axis=1)
    return np.ascontiguousarray(outT.T.astype(np.float32))
